# revision 2
# baseline (speedup 1.0000x reference)
"""Trainium2 Bass kernel for InterventionAwareStructure loss.

loss = sum_b,i,d A[b,i,d] * mask[regimes[b], d] / count   (scalar)

Data-parallel over batch across 8 NeuronCores. Each core:
  - streams its A shard [32, 512, 512] fp32 from HBM with SWDGE
    cast-DMAs (fp32 -> bf16), so SBUF only receives 16 MB instead of
    32 MB.  8 chunks of 4 batches each ride one gpsimd dma_start apiece
    (32 KB fp32 per partition line); the first and last chunks are
    split so TensorE can start early and so little PE work trails the
    final byte,
  - TensorE reduces each chunk over the source axis i with one-hot
    bf16 block stationaries (computed on HOST, DMA'd in over the
    otherwise-idle sync HWDGE ring), accumulating all 128 matmuls into
    a single [32, 512] PSUM tile.  bf16 matmuls keep the PE well ahead
    of the stream even at the mid p-state clock,
  - one final VectorE copy moves the PSUM colsums to SBUF and they are
    DMA'd out as [32, 512] fp32; the host does the tiny mask dot, the
    cross-core sum, and the divide by count.

The mask gather (256x512), the mask dot, and the final scalar
reduction are all done on host; they are negligible next to the
256 MB stream of A.
"""

import numpy as np
import ml_dtypes

import concourse.bass as bass
import concourse.tile as tile
from concourse import bacc, mybir
from concourse.bass_utils import run_bass_kernel_spmd

INTERVENTION_STRENGTH = 1.0

N_CORES = 8
B, N_REGIMES, D = 256, 16, 512
B_SH = B // N_CORES          # 32 batch items per core
NCHUNK = B_SH // 4           # 8 chunks of 4 batch items (4 MB fp32 each)
FREE = 4 * D * D // 128      # 8192 bf16 per partition per chunk

_CACHED_NC = None
_W_HOST = None


def _build_w_host() -> np.ndarray:
    """One-hot block stationary table [128, NCHUNK*32] bf16.

    Chunk g holds batches 4g..4g+3; partition p carries rows of batch
    gb = p//32.  Block g routes partition p to PSUM row 4g + p//32.
    """
    w = np.zeros((128, NCHUNK * 32), dtype=np.float32)
    for g in range(NCHUNK):
        for gb in range(4):
            w[gb * 32:(gb + 1) * 32, 32 * g + 4 * g + gb] = 1.0
    return w.astype(ml_dtypes.bfloat16)


def _build_nc() -> bass.Bass:
    nc = bacc.Bacc()
    f32 = mybir.dt.float32
    bf16 = mybir.dt.bfloat16

    a = nc.dram_tensor("a", [B_SH, D, D], f32, kind="ExternalInput")
    w = nc.dram_tensor("w", [128, NCHUNK * 32], bf16, kind="ExternalInput")
    out = nc.dram_tensor("out", [B_SH, D], f32, kind="ExternalOutput")

    # chunk g of batches (4g..4g+3) -> SBUF [128, FREE]: partition
    # p = (gb * 32 + ih) holds rows i = ih*16 + il of batch 4g+gb; free
    # axis = (il, d) with a contiguous 32 KB fp32 line per partition.
    a_view = a.rearrange(
        "(ng gb) (ih il) d -> ng (gb ih) (il d)", ng=NCHUNK, ih=32
    )

    with tile.TileContext(nc) as tc:
        with (
            tc.tile_pool(name="big", bufs=NCHUNK) as big_pool,
            tc.tile_pool(name="small", bufs=1) as small_pool,
            tc.tile_pool(name="psum", bufs=1, space="PSUM") as psum_pool,
        ):
            # Stationary table comes from host over the idle sync HWDGE
            # ring; gpsimd is busy emitting the SWDGE cast stream.
            w_t = small_pool.tile([128, NCHUNK * 32], bf16)
            nc.sync.dma_start(w_t[:], w[:, :])

            tiles = []
            for g in range(NCHUNK):
                a_t = big_pool.tile([128, FREE], bf16, tag="a")
                if g == 0:
                    # Halve the first chunk so TensorE starts ~5 us
                    # earlier than a full-chunk wait would allow.
                    h = FREE // 2
                    nc.gpsimd.dma_start(a_t[:, :h], a_view[g][:, :h])
                    nc.gpsimd.dma_start(a_t[:, h:], a_view[g][:, h:])
                elif g == NCHUNK - 1:
                    # Taper the last chunk (3 MB + 0.5 MB + 0.5 MB fp32)
                    # so only ~2 matmuls of PE work trail the last byte.
                    c0, c1 = FREE * 3 // 4, FREE * 7 // 8
                    nc.gpsimd.dma_start(a_t[:, :c0], a_view[g][:, :c0])
                    nc.gpsimd.dma_start(a_t[:, c0:c1], a_view[g][:, c0:c1])
                    nc.gpsimd.dma_start(a_t[:, c1:], a_view[g][:, c1:])
                else:
                    nc.gpsimd.dma_start(a_t[:], a_view[g])
                tiles.append(a_t)

            # All 128 matmuls accumulate into one [32, 512] PSUM tile:
            # chunk g adds rows 4g..4g+3 (its one-hot block zeroes the
            # rest), so a single start/stop group covers everything.
            ps = psum_pool.tile([B_SH, D], f32, tag="ps")
            nmm = FREE // D
            for g in range(NCHUNK):
                a_t = tiles[g]
                w_g = w_t[:, g * 32:(g + 1) * 32]
                for j in range(nmm):
                    nc.tensor.matmul(
                        ps[:], w_g, a_t[:, j * D:(j + 1) * D],
                        start=(g == 0 and j == 0),
                        stop=(g == NCHUNK - 1 and j == nmm - 1),
                    )

            o = small_pool.tile([B_SH, D], f32)
            nc.vector.tensor_copy(o[:], ps[:])
            nc.scalar.dma_start(out[:, :], o[:])

    nc.finalize()
    return nc


def _get_nc() -> bass.Bass:
    global _CACHED_NC, _W_HOST
    if _CACHED_NC is None:
        _CACHED_NC = _build_nc()
        _W_HOST = _build_w_host()
    return _CACHED_NC


def _run(a_shards, **run_kwargs):
    nc = _get_nc()
    in_maps = [
        {"a": np.ascontiguousarray(a_shards[c]), "w": _W_HOST}
        for c in range(N_CORES)
    ]
    return run_bass_kernel_spmd(nc, in_maps, list(range(N_CORES)), **run_kwargs)


def kernel(A_per_env, intervention_mask, regimes, _run_kwargs=None):
    A_per_env = np.asarray(A_per_env, dtype=np.float32)
    intervention_mask = np.asarray(intervention_mask, dtype=np.float32)
    regs = np.asarray(regimes).astype(np.int64)

    n_regimes = intervention_mask.shape[0]
    valid = regs < n_regimes
    e = np.clip(regs, 0, n_regimes - 1)
    masks = intervention_mask[e] * valid[:, None].astype(np.float32)  # [B, D]

    a_shards = [A_per_env[c * B_SH:(c + 1) * B_SH] for c in range(N_CORES)]

    res = _run(a_shards, **(_run_kwargs or {}))
    num = np.float64(0.0)
    for c in range(N_CORES):
        colsums = res.results[c]["out"].astype(np.float64)        # [32, 512]
        num += (colsums * masks[c * B_SH:(c + 1) * B_SH]).sum()

    count = masks.astype(np.float64).sum()
    loss = num / count if count > 0 else num
    out = np.asarray(INTERVENTION_STRENGTH * loss, dtype=np.float32)
    if _run_kwargs is not None:
        return out, res
    return out


# revision 14
# speedup vs baseline: 1.1384x; 1.1384x over previous
"""Trainium2 Bass kernel for InterventionAwareStructure loss.

loss = sum_b,i,d A[b,i,d] * mask[regimes[b], d] / count   (scalar)

Data-parallel over batch across 8 NeuronCores. Each core:
  - streams its A shard [32, 512, 512] fp32 from HBM on the sync HWDGE
    ring at SDMA line rate (~27 GB/s/engine x 16): 7 chunks of 4 MB
    (32 KB per partition line) and the last chunk as 4 x 1 MB quarters
    so little work trails the final byte,
  - the idle Vector / GpSimd engines (alternating per chunk) do a
    first halving reduction in place (free-axis pair add), so TensorE
    only needs 8 fp32r one-hot matmuls per chunk (5.2 us) -- safely
    faster than the 9.3 us/chunk DMA stream, which therefore never
    stalls on compute,
  - chunks 0-6 accumulate into PSUM bank A whose 28 batch rows are
    drained and DMA'd out while the last chunk is still in flight;
    only the tail quarter's add + 2 matmuls + a [4, 512] store remain
    after the last byte,
  - the one-hot stationary table is computed on HOST and DMA'd in over
    the same ring (64 KB) before the stream warms up.

The mask gather (256x512), the mask dot, and the final scalar
reduction are all done on host; they are negligible next to the
256 MB stream of A.
"""

import numpy as np

import concourse.bass as bass
import concourse.tile as tile
from concourse import bacc, mybir
from concourse.bass_utils import run_bass_kernel_spmd

INTERVENTION_STRENGTH = 1.0

N_CORES = 8
B, N_REGIMES, D = 256, 16, 512
B_SH = B // N_CORES          # 32 batch items per core
NCHUNK = B_SH // 4           # 8 chunks of 4 batch items (4 MB fp32 each)
FREE = 4 * D * D // 128      # 8192 f32 per partition per chunk
HALF = FREE // 2             # free size after the halving add
QCOL = FREE // 4             # 2048-col (1 MB) quarters for the last chunk

_CACHED_NC = None
_W_HOST = None


def _build_w_host() -> np.ndarray:
    """One-hot block stationary table [128, NCHUNK*32] fp32.

    Chunk g holds batches 4g..4g+3; partition p carries rows of batch
    gb = p//32.  Block g routes partition p to PSUM row 4g + p//32.
    """
    w = np.zeros((128, NCHUNK * 32), dtype=np.float32)
    for g in range(NCHUNK - 1):
        for gb in range(4):
            w[gb * 32:(gb + 1) * 32, 32 * g + 4 * g + gb] = 1.0
    # The tail chunk maps to PSUM rows 0-3 so its [4, 512] result can
    # be copied from the 32-partition-aligned top of its own bank.
    for gb in range(4):
        w[gb * 32:(gb + 1) * 32, 32 * (NCHUNK - 1) + gb] = 1.0
    return w


def _build_nc() -> bass.Bass:
    nc = bacc.Bacc()
    f32 = mybir.dt.float32
    f32r = mybir.dt.float32r

    a = nc.dram_tensor("a", [B_SH, D, D], f32, kind="ExternalInput")
    # fp32 bits tagged fp32r so the weights' producer dtype satisfies
    # the BIR verifier without an on-device retag copy.
    w = nc.dram_tensor("w", [128, NCHUNK * 32], f32, kind="ExternalInput").bitcast(
        f32r
    )
    out = nc.dram_tensor("out", [B_SH, D], f32, kind="ExternalOutput")

    # chunk g of batches (4g..4g+3) -> SBUF [128, FREE]: partition
    # p = (gb * 32 + ih) holds rows i = ih*16 + il of batch 4g+gb; free
    # axis = (il, d) with a contiguous 32 KB line per partition.
    a_view = a.rearrange(
        "(ng gb) (ih il) d -> ng (gb ih) (il d)", ng=NCHUNK, ih=32
    )
    # Same bytes tagged fp32r: the last two 256 KB tail pieces skip the
    # DVE add and feed matmuls directly (a DMA producer passes the
    # fp32r verifier), so almost no work trails the final byte.
    ar_view = a.bitcast(f32r).rearrange(
        "(ng gb) (ih il) d -> ng (gb ih) (il d)", ng=NCHUNK, ih=32
    )

    mult = mybir.AluOpType.mult
    add = mybir.AluOpType.add

    with tile.TileContext(nc) as tc:
        with (
            tc.tile_pool(name="big", bufs=4) as big_pool,
            tc.tile_pool(name="half", bufs=3) as half_pool,
            tc.tile_pool(name="small", bufs=1) as small_pool,
            tc.tile_pool(name="psum", bufs=2, space="PSUM") as psum_pool,
        ):
            # W rides the scalar (ACT) HWDGE ring, whose preamble also
            # finishes earlier than sync's -- so chunk 0 starts there
            # too, buying ~2 us of stream head start.
            w_t = small_pool.tile([128, NCHUNK * 32], f32r)
            nc.scalar.dma_start(w_t[:], w[:, :])

            tiles = []
            for g in range(NCHUNK - 1):
                a_t = big_pool.tile([128, FREE], f32, tag="a")
                if g == 0:
                    h = FREE // 2
                    nc.scalar.dma_start(a_t[:, :h], a_view[g][:, :h])
                    nc.scalar.dma_start(a_t[:, h:], a_view[g][:, h:])
                else:
                    nc.sync.dma_start(a_t[:], a_view[g])
                tiles.append(a_t)
            # Tail chunk: three 1 MB pieces + one 0.5 MB piece into the
            # f32 tile (DVE-added like the others), then two raw-f32r
            # 256 KB pieces that feed single matmuls directly.
            g7 = NCHUNK - 1
            a_t7 = big_pool.tile([128, FREE], f32, tag="a")
            for c0, c1 in ((0, 2048), (2048, 4096), (4096, 6144), (6144, 7168)):
                nc.sync.dma_start(a_t7[:, c0:c1], a_view[g7][:, c0:c1])
            p_ts = []
            for c0 in (7168, 7680):
                p_t = half_pool.tile([128, D], f32r, tag="p")
                nc.sync.dma_start(p_t[:], ar_view[g7][:, c0:c0 + D])
                p_ts.append(p_t)

            # First reduction level off the critical path: pair-add
            # over the free axis into an f32r half tile (the f32r
            # destination is the rounding "producer" the BIR verifier
            # wants for fp32r matmuls, and it halves TensorE's moving
            # data).  DVE runs well under the DMA pace; the big tile
            # frees as soon as the add has read it.
            halves = []
            for g in range(NCHUNK - 1):
                eng = nc.vector
                a_t = tiles[g]
                h_t = half_pool.tile([128, HALF], f32r, tag="h")
                eng.scalar_tensor_tensor(
                    out=h_t[:],
                    in0=a_t[:, :HALF],
                    scalar=1.0,
                    in1=a_t[:, HALF:],
                    op0=mult,
                    op1=add,
                )
                halves.append(h_t)

            # Chunks 0-6 accumulate into bank A (rows 0-27 of colsums);
            # it closes early so those rows stream out while the tail
            # chunk is still in flight.
            ps_a = psum_pool.tile([B_SH, D], f32, tag="psa")
            for g in range(NCHUNK - 1):
                h_t = halves[g]
                w_g = w_t[:, g * 32:(g + 1) * 32]
                for j in range(HALF // D):
                    nc.tensor.matmul(
                        ps_a[:], w_g,
                        h_t[:, j * D:(j + 1) * D],
                        start=(g == 0 and j == 0),
                        stop=(g == NCHUNK - 2 and j == HALF // D - 1),
                    )
            nbat = 4 * (NCHUNK - 1)
            o_a = small_pool.tile([nbat, D], f32)
            nc.vector.tensor_copy(o_a[:], ps_a[:nbat, :])
            nc.scalar.dma_start(out[:nbat, :], o_a[:])

            # Tail chunk into bank B (rows 0-3 via its one-hot block):
            # per-piece halving add + matmuls, then two direct-f32r
            # matmuls, so only ~1 matmul + a [4, 512] store trail the
            # last byte.
            ps_b = psum_pool.tile([B_SH, D], f32, tag="psb")
            w_g = w_t[:, g7 * 32:(g7 + 1) * 32]
            mm_b = []
            for c0, c1 in ((0, 2048), (2048, 4096), (4096, 6144), (6144, 7168)):
                qh = (c1 - c0) // 2
                q_t = half_pool.tile([128, qh], f32r, tag="q")
                nc.vector.scalar_tensor_tensor(
                    out=q_t[:],
                    in0=a_t7[:, c0:c0 + qh],
                    scalar=1.0,
                    in1=a_t7[:, c0 + qh:c1],
                    op0=mult,
                    op1=add,
                )
                for j in range(qh // D):
                    mm_b.append(q_t[:, j * D:(j + 1) * D])
            mm_b.extend(p_t[:] for p_t in p_ts)
            for k, mv in enumerate(mm_b):
                nc.tensor.matmul(
                    ps_b[:], w_g, mv,
                    start=(k == 0),
                    stop=(k == len(mm_b) - 1),
                )
            # Tail batches land in rows 0-3: a 32-partition-aligned
            # [4, 512] PSUM read, copied and stored as out rows 28-31.
            o_b = small_pool.tile([4, D], f32)
            nc.vector.tensor_copy(o_b[:], ps_b[:4, :])
            nc.scalar.dma_start(out[nbat:, :], o_b[:])

    nc.finalize()
    return nc


def _get_nc() -> bass.Bass:
    global _CACHED_NC, _W_HOST
    if _CACHED_NC is None:
        _CACHED_NC = _build_nc()
        _W_HOST = _build_w_host()
    return _CACHED_NC


def _run(a_shards, **run_kwargs):
    nc = _get_nc()
    in_maps = [
        {"a": np.ascontiguousarray(a_shards[c]), "w": _W_HOST}
        for c in range(N_CORES)
    ]
    return run_bass_kernel_spmd(nc, in_maps, list(range(N_CORES)), **run_kwargs)


def kernel(A_per_env, intervention_mask, regimes, _run_kwargs=None):
    A_per_env = np.asarray(A_per_env, dtype=np.float32)
    intervention_mask = np.asarray(intervention_mask, dtype=np.float32)
    regs = np.asarray(regimes).astype(np.int64)

    n_regimes = intervention_mask.shape[0]
    valid = regs < n_regimes
    e = np.clip(regs, 0, n_regimes - 1)
    masks = intervention_mask[e] * valid[:, None].astype(np.float32)  # [B, D]

    a_shards = [A_per_env[c * B_SH:(c + 1) * B_SH] for c in range(N_CORES)]

    res = _run(a_shards, **(_run_kwargs or {}))
    num = np.float64(0.0)
    for c in range(N_CORES):
        colsums = res.results[c]["out"].astype(np.float64)        # [32, 512]
        num += (colsums * masks[c * B_SH:(c + 1) * B_SH]).sum()

    count = masks.astype(np.float64).sum()
    loss = num / count if count > 0 else num
    out = np.asarray(INTERVENTION_STRENGTH * loss, dtype=np.float32)
    if _run_kwargs is not None:
        return out, res
    return out
